# revision 1
# baseline (speedup 1.0000x reference)
"""Two-layer GAT (DGL GATConv) on 8 Trainium2 NeuronCores.

Strategy (edge-parallel, dst-sorted):
  * Host sorts edges by dst; core c owns dst nodes [6250c, 6250(c+1)).
  * Per layer, each core projects its own nodes (x @ [Ws | Ws@bd(al) | Wd@bd(ar)]
    -> per-node table rows [fs | el] plus er), all-gathers the table, then
    processes its own edges grouped by 128-dst-node windows:
      - dma_gather of 512B table rows by src (two gathers: table split in two
        halves so indices fit int16),
      - onehot[j, m] = (dst_local[j] == m) via one is_equal per batch,
      - onehotT[m, j] likewise from a broadcast-replicated dst_local row;
        er_edge[j, h] = onehotT.T @ er_window via TensorE,
      - ee = exp(leaky_relu(el + er_edge)); msg = [ee * fs | ee],
      - seg[m, :] += onehot.T @ msg accumulated in PSUM over the group's
        chunks; last columns give the softmax denominator.
  * Layer-1 epilogue: h1 = elu(seg/denom + b1) kept in SBUF.
  * Layer-2 epilogue: mean over heads + log_softmax (Ln deferred to one pass).

Softmax max-subtraction is skipped: e = lrelu(el+er) with the given scales is
|e| < ~10, well within fp32 exp range, and alpha is shift-invariant.

Host work is index/schedule preparation only (sort, counts, layouts) plus
constant reparameterization (W @ blockdiag(a), np.tile of biases).
"""

import math
import os
import sys
from contextlib import ExitStack

import numpy as np

for _p in ("/opt/trn_rl_repo", "/root/.axon_site/_ro/trn_rl_repo"):
    if os.path.isdir(_p) and _p not in sys.path:
        sys.path.append(_p)

import ml_dtypes

import concourse.bass as bass
import concourse.tile as tile
from concourse import bacc, mybir
from concourse.bass_utils import run_bass_kernel_spmd

BF16 = ml_dtypes.bfloat16

N = 50000
E = 800000
F_IN = 128
H, D, C = 4, 32, 47
HD = H * D            # 128
HC = H * C            # 188
NEG_SLOPE = 0.2

NCORES = 8
P = 128
NPC = N // NCORES         # 6250
G = math.ceil(NPC / P)    # 49
NPAD = G * P              # 6272
NTAB = NPAD * NCORES      # 50176
S_SPLIT = NPAD * (NCORES // 2)   # 25088 (< int16 max on both halves)

W1 = HD + H               # 132
W2 = HC + H               # 192
ELEM = 256                # table row width (bf16) -> 512B, dma_gather aligned
OOR = 200.0               # out-of-window dst_local marker
BATCH = 8

LAST_EXEC_NS = None


def _schedule(src, dst):
    """Build per-core gather/one-hot schedule. Returns dict of arrays + KA/KB."""
    order = np.argsort(dst, kind="stable")
    s_src = src[order].astype(np.int64)
    s_dst = dst[order].astype(np.int64)

    core_of = s_dst // NPC
    g_of = (s_dst % NPC) // P
    pg_src = s_src + (s_src // NPC) * (NPAD - NPC)   # padded-global src row
    half = (pg_src >= S_SPLIT).astype(np.int64)
    win = (s_dst % NPC) % P

    cgh = (core_of * G + g_of) * 2 + half
    order2 = np.argsort(cgh, kind="stable")
    cgh = cgh[order2]
    pg_src = pg_src[order2]
    win = win[order2]

    counts = np.bincount(cgh, minlength=NCORES * G * 2).reshape(NCORES, G, 2)
    KA = int(math.ceil(counts[:, :, 0].max() / P))
    KB = int(math.ceil(counts[:, :, 1].max() / P))
    K = KA + KB

    starts = np.zeros(NCORES * G * 2 + 1, dtype=np.int64)
    np.cumsum(counts.ravel(), out=starts[1:])
    pos_in_run = np.arange(len(cgh)) - starts[cgh]

    # flat slot position within the [K*128] group stream
    base = np.where(cgh % 2 == 0, 0, KA * P)
    flat = base + pos_in_run
    cg = cgh // 2

    idx_flat = np.zeros((NCORES * G, K * P), dtype=np.int64)   # dummy row 0
    dstl_flat = np.full((NCORES * G, K * P), OOR, dtype=np.float32)
    idx_val = np.where(cgh % 2 == 0, pg_src, pg_src - S_SPLIT)
    idx_flat[cg, flat] = idx_val
    dstl_flat[cg, flat] = win

    idx_flat = idx_flat.reshape(NCORES, G, K, P)
    dstl_flat = dstl_flat.reshape(NCORES, G, K, P)

    def wrap(a):
        # [.., n] flat slot-major -> [.., 128, n/16] wrapped+replicated
        n = a.shape[-1]
        w = a.reshape(*a.shape[:-1], n // 16, 16)
        w = np.swapaxes(w, -1, -2)                    # [16, n/16]
        return np.tile(w, (1, 1, 8, 1)).astype(np.int16)  # [128, n/16]

    idxA_w = wrap(idx_flat[:, :, :KA, :].reshape(NCORES, G, KA * P))
    idxB_w = wrap(idx_flat[:, :, KA:, :].reshape(NCORES, G, KB * P))

    dstl_row = dstl_flat.reshape(NCORES, G, K * P).astype(BF16)
    dstl_col = np.swapaxes(dstl_flat, 2, 3).astype(BF16)      # [NC, G, 128, K]
    return dict(idxA_w=idxA_w, idxB_w=idxB_w, dstl_row=dstl_row,
                dstl_col=dstl_col, KA=KA, KB=KB)


def _blockdiag(a, hd, h, dim):
    out = np.zeros((hd, h), dtype=np.float32)
    for i in range(h):
        out[i * dim:(i + 1) * dim, i] = a[i]
    return out


_PHASES = ["proj1", "ag1", "edges1", "proj2", "edges2", "final", "full"]


def _build_program(KA, KB):
    K = KA + KB
    phase = os.environ.get("GAT_PHASE", "full")

    def go(p):
        # build phase p if the requested phase is at or beyond p
        return _PHASES.index(phase) >= _PHASES.index(p)
    nc = bacc.Bacc("TRN2", target_bir_lowering=False, debug=False,
                   num_devices=NCORES)
    dt = mybir.dt
    f32, bf16, i16 = dt.float32, dt.bfloat16, dt.int16

    def inp(name, shape, d=f32):
        return nc.dram_tensor(name, shape, d, kind="ExternalInput").ap()

    x_own = inp("x_own", [NPAD, F_IN])
    w1cat = inp("w1cat", [F_IN, W1 + H], bf16)
    w2cat = inp("w2cat", [F_IN, W2 + H], bf16)
    b1_t = inp("b1_t", [P, HD])
    b2m_t = inp("b2m_t", [P, C])
    iota_r = inp("iota_r", [P, P], bf16)       # iota_r[p, m] = m
    iota_c = inp("iota_c", [P, 1])             # iota_c[p, 0] = p (f32)
    ident_t = inp("ident_t", [P, P])
    idxA_in = inp("idxA_in", [G, P, KA * 8], i16)
    idxB_in = inp("idxB_in", [G, P, KB * 8], i16)
    dstl_row_in = inp("dstl_row_in", [G, K * P], bf16)
    dstl_col_in = inp("dstl_col_in", [G, P, K], bf16)

    y_out = nc.dram_tensor("y_out", [NPAD, C], f32, kind="ExternalOutput").ap()
    dbg = nc.dram_tensor("dbg", [NTAB, ELEM], bf16, kind="ExternalOutput").ap()

    tab1_own = nc.dram_tensor("tab1_own", [NPAD, ELEM], bf16).ap()
    tab1 = nc.dram_tensor("tab1", [NTAB, ELEM], bf16, addr_space="Shared").ap()
    tab1B = nc.dram_tensor("tab1B", [NTAB - S_SPLIT, ELEM], bf16).ap()
    er1_d = nc.dram_tensor("er1_d", [NPAD, H], f32).ap()
    tab2_own = nc.dram_tensor("tab2_own", [NPAD, ELEM], bf16).ap()
    tab2 = nc.dram_tensor("tab2", [NTAB, ELEM], bf16, addr_space="Shared").ap()
    tab2B = nc.dram_tensor("tab2B", [NTAB - S_SPLIT, ELEM], bf16).ap()
    er2_d = nc.dram_tensor("er2_d", [NPAD, H], f32).ap()

    with tile.TileContext(nc) as tc, ExitStack() as ctx:
        const = ctx.enter_context(tc.tile_pool(name="const", bufs=1))
        sb = ctx.enter_context(tc.tile_pool(name="sb", bufs=3))
        gat = ctx.enter_context(tc.tile_pool(name="gat", bufs=2))
        ps = ctx.enter_context(tc.tile_pool(name="ps", bufs=2, space="PSUM"))
        psg = ctx.enter_context(tc.tile_pool(name="psg", bufs=2, space="PSUM"))
        big = ctx.enter_context(tc.tile_pool(name="big", bufs=1))

        noconst = os.environ.get("GAT_NOCONST")
        iota = const.tile([P, P], bf16)
        nc.sync.dma_start(iota[:], iota_r[:])
        iotac = const.tile([P, 1], f32)
        nc.sync.dma_start(iotac[:], iota_c[:])
        ident = b1s = b2ms = w1 = w2 = None
        if not noconst:
            ident = const.tile([P, P], f32)
            nc.sync.dma_start(ident[:], ident_t[:])
            b1s = const.tile([P, HD], f32)
            nc.sync.dma_start(b1s[:], b1_t[:])
            b2ms = const.tile([P, C], f32)
            nc.sync.dma_start(b2ms[:], b2m_t[:])
            w1 = const.tile([P, W1 + H], bf16)
            nc.sync.dma_start(w1[:], w1cat[:])
            w2 = const.tile([P, W2 + H], bf16)
            nc.sync.dma_start(w2[:], w2cat[:])

        h1 = zs = ss = None
        if not os.environ.get("GAT_NOBIG"):
            h1 = big.tile([P, G, F_IN], f32)
            zs = big.tile([P, G, C], f32)
            ss = big.tile([P, G], f32)

        # ---------------- projection ----------------
        def project(src_tile_of, wcat, width, tab_own_d, er_d):
            for g in range(G):
                xt = src_tile_of(g)
                xT_ps = ps.tile([F_IN, P], f32, space="PSUM", tag="xT_ps")
                nc.tensor.transpose(xT_ps[:], xt[:], ident[:])
                xT = sb.tile([F_IN, P], bf16, tag="xT")
                nc.vector.tensor_copy(xT[:], xT_ps[:])
                pr = ps.tile([P, width + H], f32, space="PSUM", tag="proj")
                nc.tensor.matmul(pr[:], lhsT=xT[:], rhs=wcat[:, :width + H],
                                 start=True, stop=True)
                tb = sb.tile([P, width], bf16, tag="tabrow")
                nc.vector.tensor_copy(tb[:], pr[:, :width])
                nc.sync.dma_start(tab_own_d[g * P:(g + 1) * P, :width], tb[:])
                er = sb.tile([P, H], f32, tag="errow")
                nc.vector.tensor_copy(er[:], pr[:, width:width + H])
                nc.sync.dma_start(er_d[g * P:(g + 1) * P, :], er[:])

        def x_tile(g):
            t = sb.tile([P, F_IN], f32, tag="xload")
            nc.sync.dma_start(t[:], x_own[g * P:(g + 1) * P, :])
            return t

        if os.environ.get("GAT_NOPROJ"):
            for g in range(G):
                nc.gpsimd.dma_start(out=tab1_own[g * P:(g + 1) * P, :F_IN],
                                    in_=x_own[g * P:(g + 1) * P, :])
                nc.sync.dma_start(out=er1_d[g * P:(g + 1) * P, :],
                                  in_=x_own[g * P:(g + 1) * P, :H])
        else:
            project(x_tile, w1, W1, tab1_own, er1_d)

        if phase == "proj1":
            nc.sync.dma_start(dbg[:NPAD, :], tab1_own[:])
        if go("ag1"):
            nc.gpsimd.collective_compute(
            "AllGather", mybir.AluOpType.bypass,
                replica_groups=[list(range(NCORES))],
                ins=[tab1_own[:]], outs=[tab1[:]])
            nc.sync.dma_start(tab1B[:], tab1[S_SPLIT:, :])
        if phase == "ag1":
            nc.sync.dma_start(dbg[:], tab1[:])

        # ---------------- edge phase ----------------
        edgelvl = int(os.environ.get("GAT_EDGELVL", "9"))

        def edge_phase(tab_full, tab_B, er_d, width, out_cb):
            nb = math.ceil(K / BATCH)
            for g in range(G):
                idxA_t = sb.tile([P, KA * 8], i16, tag="idxA")
                nc.sync.dma_start(idxA_t[:], idxA_in[g])
                idxB_t = sb.tile([P, KB * 8], i16, tag="idxB")
                nc.sync.dma_start(idxB_t[:], idxB_in[g])
                dcol = sb.tile([P, K], bf16, tag="dcol")
                nc.sync.dma_start(dcol[:], dstl_col_in[g])
                drep = sb.tile([P, K * P], bf16, tag="drep")
                if os.environ.get("GAT_NOBCAST"):
                    nc.sync.dma_start(drep[:1, :], dstl_row_in[g:g + 1, :])
                else:
                    nc.sync.dma_start(
                        drep[:], dstl_row_in[g:g + 1, :].to_broadcast([P, K * P]))
                erw_f = sb.tile([P, H], f32, tag="erwf")
                nc.sync.dma_start(erw_f[:], er_d[g * P:(g + 1) * P, :])
                erw = sb.tile([P, H], bf16, tag="erw")
                nc.vector.tensor_copy(erw[:], erw_f[:])

                gt = gat.tile([P, K, ELEM], bf16, tag="gt")
                if edgelvl >= 1:
                    nc.gpsimd.dma_gather(
                        out_ap=gt[:, :KA, :], in_ap=tab_full[:S_SPLIT, :],
                        idxs_ap=idxA_t[:], num_idxs=KA * P,
                        num_idxs_reg=KA * P, elem_size=ELEM,
                        single_packet=False)
                    nc.gpsimd.dma_gather(
                        out_ap=gt[:, KA:, :], in_ap=tab_B[:],
                        idxs_ap=idxB_t[:], num_idxs=KB * P,
                        num_idxs_reg=KB * P, elem_size=ELEM,
                        single_packet=False)

                seg = None
                if not os.environ.get("GAT_NOSEG"):
                    seg = psg.tile([P, width], f32, space="PSUM", tag="seg")
                for b in range(nb):
                    k0 = b * BATCH
                    kb = min(BATCH, K - k0)
                    if edgelvl < 2:
                        continue
                    oh = sb.tile([P, BATCH, P], bf16, tag="oh")
                    nc.vector.tensor_tensor(
                        out=oh[:, :kb, :],
                        in0=dcol[:, k0:k0 + kb, None].to_broadcast([P, kb, P]),
                        in1=iota[:, None, :].to_broadcast([P, kb, P]),
                        op=mybir.AluOpType.is_equal)
                    if edgelvl < 3:
                        continue
                    ohT = sb.tile([P, BATCH, P], bf16, tag="ohT")
                    nc.vector.tensor_scalar(
                        out=ohT[:, :kb, :],
                        in0=drep[:, k0 * P:(k0 + kb) * P].rearrange(
                            "p (k j) -> p k j", k=kb),
                        scalar1=iotac[:, :1], scalar2=None,
                        op0=mybir.AluOpType.is_equal)
                    if edgelvl < 4:
                        continue
                    erp = psg.tile([P, BATCH, H], f32, space="PSUM", tag="erp")
                    for c in range(kb):
                        nc.tensor.matmul(
                            erp[:, c, :], lhsT=ohT[:, c, :], rhs=erw[:],
                            start=True, stop=True)
                    if edgelvl < 5:
                        continue
                    ev = sb.tile([P, BATCH, H], f32, tag="ev")
                    nc.vector.tensor_tensor(
                        out=ev[:, :kb, :],
                        in0=gt[:, k0:k0 + kb, width - H:width],
                        in1=erp[:, :kb, :], op=mybir.AluOpType.add)
                    nc.vector.scalar_tensor_tensor(
                        out=ev[:, :kb, :], in0=ev[:, :kb, :], scalar=NEG_SLOPE,
                        in1=ev[:, :kb, :],
                        op0=mybir.AluOpType.mult, op1=mybir.AluOpType.max)
                    if edgelvl < 6:
                        continue
                    mt = sb.tile([P, BATCH, width], bf16, tag="mt")
                    nc.scalar.activation(
                        mt[:, :kb, width - H:width], ev[:, :kb, :],
                        mybir.ActivationFunctionType.Exp)
                    nc.vector.tensor_tensor(
                        out=mt[:, :kb, :width - H].rearrange(
                            "p k (h d) -> p k h d", h=H),
                        in0=gt[:, k0:k0 + kb, :width - H].rearrange(
                            "p k (h d) -> p k h d", h=H),
                        in1=mt[:, :kb, width - H:width, None].to_broadcast(
                            [P, kb, H, (width - H) // H]),
                        op=mybir.AluOpType.mult)
                    if edgelvl < 7:
                        continue
                    for c in range(kb):
                        nc.tensor.matmul(
                            seg[:], lhsT=oh[:, c, :], rhs=mt[:, c, :],
                            start=(b == 0 and c == 0),
                            stop=(b == nb - 1 and c == kb - 1))
                if edgelvl >= 7:
                    out_cb(g, seg)

        def l1_out(g, seg):
            dn = sb.tile([P, H], f32, tag="dn")
            nc.vector.tensor_scalar_max(dn[:], seg[:, HD:HD + H], 1e-30)
            rd = sb.tile([P, H], f32, tag="rd")
            nc.vector.reciprocal(rd[:], dn[:])
            ht = sb.tile([P, F_IN], f32, tag="ht")
            nc.vector.tensor_tensor(
                out=ht[:].rearrange("p (h d) -> p h d", h=H),
                in0=seg[:, :HD].rearrange("p (h d) -> p h d", h=H),
                in1=rd[:, :, None].to_broadcast([P, H, D]),
                op=mybir.AluOpType.mult)
            nc.vector.tensor_tensor(
                out=ht[:], in0=ht[:], in1=b1s[:], op=mybir.AluOpType.add)
            mn = sb.tile([P, F_IN], f32, tag="mn")
            nc.vector.tensor_scalar_min(mn[:], ht[:], 0.0)
            nc.scalar.activation(mn[:], mn[:], mybir.ActivationFunctionType.Exp)
            nc.vector.scalar_tensor_tensor(
                out=h1[:, g, :], in0=mn[:], scalar=-1.0, in1=ht[:],
                op0=mybir.AluOpType.add, op1=mybir.AluOpType.max)

        if go("edges1"):
            edge_phase(tab1, tab1B, er1_d, W1, l1_out)
        if phase == "edges1" and edgelvl >= 7:
            for g in range(G):
                hd_t = sb.tile([P, F_IN], bf16, tag="hdump")
                nc.vector.tensor_copy(hd_t[:], h1[:, g, :])
                nc.sync.dma_start(dbg[g * P:(g + 1) * P, :F_IN], hd_t[:])

        def h1_tile(g):
            return h1[:, g, :]

        if go("proj2"):
            project(h1_tile, w2, W2, tab2_own, er2_d)
            nc.gpsimd.collective_compute(
                "AllGather", mybir.AluOpType.bypass,
                replica_groups=[list(range(NCORES))],
                ins=[tab2_own[:]], outs=[tab2[:]])
            nc.sync.dma_start(tab2B[:], tab2[S_SPLIT:, :])

        def l2_out(g, seg):
            dn = sb.tile([P, H], f32, tag="dn2")
            nc.vector.tensor_scalar_max(dn[:], seg[:, HC:HC + H], 1e-30)
            rd = sb.tile([P, H], f32, tag="rd2")
            nc.vector.reciprocal(rd[:], dn[:])
            nc.vector.tensor_scalar_mul(rd[:], rd[:], 1.0 / H)
            z = sb.tile([P, HC], f32, tag="z")
            nc.vector.tensor_tensor(
                out=z[:].rearrange("p (h c) -> p h c", h=H),
                in0=seg[:, :HC].rearrange("p (h c) -> p h c", h=H),
                in1=rd[:, :, None].to_broadcast([P, H, C]),
                op=mybir.AluOpType.mult)
            z4 = sb.tile([P, C], f32, tag="z4")
            nc.vector.reduce_sum(
                z4[:], z[:].rearrange("p (h c) -> p c h", h=H),
                axis=mybir.AxisListType.X)
            nc.vector.tensor_tensor(
                out=z4[:], in0=z4[:], in1=b2ms[:], op=mybir.AluOpType.add)
            zm = sb.tile([P, 1], f32, tag="zm")
            nc.vector.reduce_max(zm[:], z4[:], axis=mybir.AxisListType.X)
            nc.vector.tensor_scalar(
                out=zs[:, g, :], in0=z4[:], scalar1=zm[:, :1], scalar2=None,
                op0=mybir.AluOpType.subtract)
            es = sb.tile([P, C], f32, tag="es")
            nc.scalar.activation(es[:], zs[:, g, :],
                                 mybir.ActivationFunctionType.Exp,
                                 accum_out=ss[:, g:g + 1])

        if go("edges2"):
            edge_phase(tab2, tab2B, er2_d, W2, l2_out)

        if go("final"):
            lg = sb.tile([P, G], f32, tag="lg")
            nc.scalar.activation(lg[:], ss[:], mybir.ActivationFunctionType.Ln)
            for g in range(G):
                yt = sb.tile([P, C], f32, tag="yt")
                nc.vector.tensor_scalar(
                    out=yt[:], in0=zs[:, g, :], scalar1=lg[:, g:g + 1],
                    scalar2=None, op0=mybir.AluOpType.subtract)
                nc.sync.dma_start(y_out[g * P:(g + 1) * P, :], yt[:])

    nc.compile()
    return nc


def kernel(x, src, dst, W1s, W1d, al1, ar1, b1, W2s, W2d, al2, ar2, b2):
    global LAST_EXEC_NS
    x = np.asarray(x, dtype=np.float32)
    src = np.asarray(src, dtype=np.int32)
    dst = np.asarray(dst, dtype=np.int32)

    sch = _schedule(src, dst)
    KA, KB = sch["KA"], sch["KB"]

    def wcat(Ws, Wd, al, ar, hd, h, dim):
        Ws = np.asarray(Ws, np.float32)
        Wd = np.asarray(Wd, np.float32)
        wel = Ws @ _blockdiag(np.asarray(al, np.float32), hd, h, dim)
        wer = Wd @ _blockdiag(np.asarray(ar, np.float32), hd, h, dim)
        return np.concatenate([Ws, wel, wer], axis=1)

    w1c = wcat(W1s, W1d, al1, ar1, HD, H, D).astype(BF16)
    w2c = wcat(W2s, W2d, al2, ar2, HC, H, C).astype(BF16)

    iota_r = np.tile(np.arange(P, dtype=np.float32), (P, 1)).astype(BF16)
    iota_c = np.arange(P, dtype=np.float32)[:, None]
    ident_np = np.eye(P, dtype=np.float32)
    b1_np = np.tile(np.asarray(b1, np.float32)[None, :], (P, 1))
    b2m_np = np.tile(np.asarray(b2, np.float32).reshape(H, C).mean(0)[None, :],
                     (P, 1))

    x_pad = np.zeros((NCORES, NPAD, F_IN), np.float32)
    x_pad[:, :NPC, :] = x.reshape(NCORES, NPC, F_IN)

    nc = _build_program(KA, KB)

    in_maps = []
    for c in range(NCORES):
        in_maps.append({
            "x_own": x_pad[c],
            "w1cat": w1c, "w2cat": w2c,
            "b1_t": b1_np, "b2m_t": b2m_np,
            "iota_r": iota_r, "iota_c": iota_c, "ident_t": ident_np,
            "idxA_in": sch["idxA_w"][c], "idxB_in": sch["idxB_w"][c],
            "dstl_row_in": sch["dstl_row"][c],
            "dstl_col_in": sch["dstl_col"][c],
        })

    res = run_bass_kernel_spmd(nc, in_maps, list(range(NCORES)),
                               trace=bool(os.environ.get("GAT_TRACE")))
    LAST_EXEC_NS = res.exec_time_ns
    out = np.concatenate(
        [res.results[c]["y_out"][:NPC] for c in range(NCORES)], axis=0)
    return out.astype(np.float32)



# revision 11
# speedup vs baseline: 1.2609x; 1.2609x over previous
"""Two-layer GAT (DGL GATConv) on 8 Trainium2 NeuronCores.

Strategy (edge-parallel, dst-sorted):
  * Host sorts edges by dst; core c owns dst nodes [6250c, 6250(c+1)).
  * Per layer, each core projects its own nodes (x @ [Ws | Ws@bd(al) | Wd@bd(ar)]
    -> per-node table rows [fs | el] plus er), all-gathers the table, then
    processes its own edges grouped by 128-dst-node windows:
      - dma_gather of 512B table rows by src (two gathers: table split in two
        halves so indices fit int16),
      - onehot[j, m] = (dst_local[j] == m) via one is_equal per batch,
      - onehotT[m, j] likewise from a broadcast-replicated dst_local row;
        er_edge[j, h] = onehotT.T @ er_window via TensorE,
      - ee = exp(leaky_relu(el + er_edge)); msg = [ee * fs | ee],
      - seg[m, :] += onehot.T @ msg accumulated in PSUM over the group's
        chunks; last columns give the softmax denominator.
  * Layer-1 epilogue: h1 = elu(seg/denom + b1) kept in SBUF.
  * Layer-2 epilogue: mean over heads + log_softmax (Ln deferred to one pass).

Softmax max-subtraction is skipped: e = lrelu(el+er) with the given scales is
|e| < ~10, well within fp32 exp range, and alpha is shift-invariant.

Host work is index/schedule preparation only (sort, counts, layouts) plus
constant reparameterization (W @ blockdiag(a), np.tile of biases).
"""

import math
import os
import sys
from contextlib import ExitStack

import numpy as np

for _p in ("/opt/trn_rl_repo", "/root/.axon_site/_ro/trn_rl_repo"):
    if os.path.isdir(_p) and _p not in sys.path:
        sys.path.append(_p)

import ml_dtypes

import concourse.bass as bass
import concourse.tile as tile
from concourse import bacc, mybir
from concourse.bass_utils import run_bass_kernel_spmd

BF16 = ml_dtypes.bfloat16

N = 50000
E = 800000
F_IN = 128
H, D, C = 4, 32, 47
HD = H * D            # 128
HC = H * C            # 188
NEG_SLOPE = 0.2

NCORES = 8
P = 128
NPC = N // NCORES         # 6250
G = math.ceil(NPC / P)    # 49
NPAD = G * P              # 6272
NTAB = NPAD * NCORES      # 50176
S_SPLIT = NPAD * (NCORES // 2)   # 25088 (< int16 max on both halves)

W1 = HD + H               # 132
W2 = HC + H               # 192
ELEM = 256                # table row width (bf16) -> 512B, dma_gather aligned
OOR = 200.0               # out-of-window dst_local marker
BATCH = 8

LAST_EXEC_NS = None


def _schedule(src, dst):
    """Build per-core gather/one-hot schedule. Returns dict of arrays + KA/KB."""
    order = np.argsort(dst, kind="stable")
    s_src = src[order].astype(np.int64)
    s_dst = dst[order].astype(np.int64)

    core_of = s_dst // NPC
    g_of = (s_dst % NPC) // P
    pg_src = s_src + (s_src // NPC) * (NPAD - NPC)   # padded-global src row
    half = (pg_src >= S_SPLIT).astype(np.int64)
    win = (s_dst % NPC) % P

    cgh = (core_of * G + g_of) * 2 + half
    order2 = np.argsort(cgh, kind="stable")
    cgh = cgh[order2]
    pg_src = pg_src[order2]
    win = win[order2]

    counts = np.bincount(cgh, minlength=NCORES * G * 2).reshape(NCORES, G, 2)
    KA = int(math.ceil(counts[:, :, 0].max() / P))
    KB = int(math.ceil(counts[:, :, 1].max() / P))
    K = KA + KB

    starts = np.zeros(NCORES * G * 2 + 1, dtype=np.int64)
    np.cumsum(counts.ravel(), out=starts[1:])
    pos_in_run = np.arange(len(cgh)) - starts[cgh]

    # flat slot position within the [K*128] group stream
    base = np.where(cgh % 2 == 0, 0, KA * P)
    flat = base + pos_in_run
    cg = cgh // 2

    idx_flat = np.zeros((NCORES * G, K * P), dtype=np.int64)   # dummy row 0
    dstl_flat = np.full((NCORES * G, K * P), OOR, dtype=np.float32)
    idx_val = np.where(cgh % 2 == 0, pg_src, pg_src - S_SPLIT)
    idx_flat[cg, flat] = idx_val
    dstl_flat[cg, flat] = win

    idx_flat = idx_flat.reshape(NCORES, G, K, P)
    dstl_flat = dstl_flat.reshape(NCORES, G, K, P)

    def wrap(a):
        # [.., n] flat slot-major -> [.., 128, n/16] wrapped+replicated
        n = a.shape[-1]
        w = a.reshape(*a.shape[:-1], n // 16, 16)
        w = np.swapaxes(w, -1, -2)                    # [16, n/16]
        return np.tile(w, (1, 1, 8, 1)).astype(np.int16)  # [128, n/16]

    idxA_w = wrap(idx_flat[:, :, :KA, :].reshape(NCORES, G, KA * P))
    idxB_w = wrap(idx_flat[:, :, KA:, :].reshape(NCORES, G, KB * P))

    dstl_row = dstl_flat.reshape(NCORES, G, K * P).astype(BF16)
    dstl_col = np.swapaxes(dstl_flat, 2, 3).astype(BF16)      # [NC, G, 128, K]
    return dict(idxA_w=idxA_w, idxB_w=idxB_w, dstl_row=dstl_row,
                dstl_col=dstl_col, KA=KA, KB=KB)


def _blockdiag(a, hd, h, dim):
    out = np.zeros((hd, h), dtype=np.float32)
    for i in range(h):
        out[i * dim:(i + 1) * dim, i] = a[i]
    return out


_PHASES = ["proj1", "ag1", "edges1", "proj2", "edges2", "final", "full"]


def _build_program(KA, KB):
    K = KA + KB
    phase = os.environ.get("GAT_PHASE", "full")

    def go(p):
        # build phase p if the requested phase is at or beyond p
        return _PHASES.index(phase) >= _PHASES.index(p)
    nc = bacc.Bacc("TRN2", target_bir_lowering=False, debug=False,
                   num_devices=NCORES, num_swdge_queues=4)
    dt = mybir.dt
    f32, bf16, i16 = dt.float32, dt.bfloat16, dt.int16

    def inp(name, shape, d=f32):
        return nc.dram_tensor(name, shape, d, kind="ExternalInput").ap()

    x_own = inp("x_own", [NPAD, F_IN])
    w1cat = inp("w1cat", [F_IN, W1 + H], bf16)
    w2cat = inp("w2cat", [F_IN, W2 + H], bf16)
    b1_t = inp("b1_t", [P, HD])
    b2m_t = inp("b2m_t", [P, C])
    iota_r = inp("iota_r", [P, P], bf16)       # iota_r[p, m] = m
    iota_c = inp("iota_c", [P, 1])             # iota_c[p, 0] = p (f32)
    iotac_rep_in = inp("iotac_rep", [P, K * P], bf16)  # [p, j] = p
    ident_t = inp("ident_t", [P, P])
    idxA_in = inp("idxA_in", [G, P, KA * 8], i16)
    idxB_in = inp("idxB_in", [G, P, KB * 8], i16)
    dstl_row_in = inp("dstl_row_in", [G, K * P], bf16)
    dstl_col_in = inp("dstl_col_in", [G, P, K], bf16)

    y_out = nc.dram_tensor("y_out", [NPAD, C], f32, kind="ExternalOutput").ap()
    dbg = nc.dram_tensor("dbg", [NTAB, ELEM], bf16, kind="ExternalOutput").ap()

    tab1_own = nc.dram_tensor("tab1_own", [NPAD, ELEM], bf16).ap()
    tab1 = nc.dram_tensor("tab1", [NTAB, ELEM], bf16, addr_space="Shared").ap()
    tab1B = nc.dram_tensor("tab1B", [NTAB - S_SPLIT, ELEM], bf16).ap()
    er1_d = nc.dram_tensor("er1_d", [NPAD, H], f32).ap()
    tab2_own = nc.dram_tensor("tab2_own", [NPAD, ELEM], bf16).ap()
    tab2 = nc.dram_tensor("tab2", [NTAB, ELEM], bf16, addr_space="Shared").ap()
    tab2B = nc.dram_tensor("tab2B", [NTAB - S_SPLIT, ELEM], bf16).ap()
    er2_d = nc.dram_tensor("er2_d", [NPAD, H], f32).ap()

    with tile.TileContext(nc) as tc, ExitStack() as ctx:
        const = ctx.enter_context(tc.tile_pool(name="const", bufs=1))
        sb = ctx.enter_context(tc.tile_pool(name="sb", bufs=3))
        gat = ctx.enter_context(tc.tile_pool(name="gat", bufs=2))
        ps = ctx.enter_context(tc.tile_pool(name="ps", bufs=2, space="PSUM"))
        psg = ctx.enter_context(tc.tile_pool(name="psg", bufs=2, space="PSUM"))
        big = ctx.enter_context(tc.tile_pool(name="big", bufs=1))

        noconst = os.environ.get("GAT_NOCONST")
        iota = const.tile([P, P], bf16)
        nc.sync.dma_start(iota[:], iota_r[:])
        iotac = const.tile([P, 1], f32)
        nc.sync.dma_start(iotac[:], iota_c[:])
        iotac_rep = const.tile([P, K * P], bf16)
        nc.sync.dma_start(iotac_rep[:], iotac_rep_in[:])
        ident = b1s = b2ms = w1 = w2 = None
        if not noconst:
            ident = const.tile([P, P], f32)
            nc.sync.dma_start(ident[:], ident_t[:])
            b1s = const.tile([P, HD], f32)
            nc.sync.dma_start(b1s[:], b1_t[:])
            b2ms = const.tile([P, C], f32)
            nc.sync.dma_start(b2ms[:], b2m_t[:])
            w1 = const.tile([P, W1 + H], bf16)
            nc.sync.dma_start(w1[:], w1cat[:])
            w2 = const.tile([P, W2 + H], bf16)
            nc.sync.dma_start(w2[:], w2cat[:])

        h1 = zs = ss = None
        if not os.environ.get("GAT_NOBIG"):
            h1 = big.tile([P, G, F_IN], f32)
            zs = big.tile([P, G, C], f32)
            ss = big.tile([P, G], f32)

        # ---------------- projection ----------------
        def project(src_tile_of, wcat, width, tab_own_d, er_d):
            for g in range(G):
                xt = src_tile_of(g)
                xT_ps = ps.tile([F_IN, P], f32, space="PSUM", tag="xT_ps")
                nc.tensor.transpose(xT_ps[:], xt[:], ident[:])
                xT = sb.tile([F_IN, P], bf16, tag="xT")
                nc.vector.tensor_copy(xT[:], xT_ps[:])
                pr = ps.tile([P, width + H], f32, space="PSUM", tag="proj")
                nc.tensor.matmul(pr[:], lhsT=xT[:], rhs=wcat[:, :width + H],
                                 start=True, stop=True)
                tb = sb.tile([P, width], bf16, tag="tabrow")
                nc.vector.tensor_copy(tb[:], pr[:, :width])
                nc.sync.dma_start(tab_own_d[g * P:(g + 1) * P, :width], tb[:])
                er = sb.tile([P, H], f32, tag="errow")
                nc.vector.tensor_copy(er[:], pr[:, width:width + H])
                nc.sync.dma_start(er_d[g * P:(g + 1) * P, :], er[:])

        def x_tile(g):
            t = sb.tile([P, F_IN], f32, tag="xload")
            nc.sync.dma_start(t[:], x_own[g * P:(g + 1) * P, :])
            return t

        if os.environ.get("GAT_NOPROJ"):
            for g in range(G):
                nc.gpsimd.dma_start(out=tab1_own[g * P:(g + 1) * P, :F_IN],
                                    in_=x_own[g * P:(g + 1) * P, :])
                nc.sync.dma_start(out=er1_d[g * P:(g + 1) * P, :],
                                  in_=x_own[g * P:(g + 1) * P, :H])
        else:
            project(x_tile, w1, W1, tab1_own, er1_d)

        if phase == "proj1":
            nc.sync.dma_start(dbg[:NPAD, :], tab1_own[:])
        if go("ag1"):
            nc.gpsimd.collective_compute(
            "AllGather", mybir.AluOpType.bypass,
                replica_groups=[list(range(NCORES))],
                ins=[tab1_own[:]], outs=[tab1[:]])
            nc.sync.dma_start(tab1B[:], tab1[S_SPLIT:, :])
        if phase == "ag1":
            nc.sync.dma_start(dbg[:], tab1[:])

        # ---------------- edge phase ----------------
        edgelvl = int(os.environ.get("GAT_EDGELVL", "9"))

        def edge_phase(tab_full, tab_B, er_d, width, out_cb):
            nb = math.ceil(K / BATCH)
            for g in range(G):
                idxA_t = sb.tile([P, KA * 8], i16, tag="idxA")
                nc.sync.dma_start(idxA_t[:], idxA_in[g])
                idxB_t = sb.tile([P, KB * 8], i16, tag="idxB")
                nc.sync.dma_start(idxB_t[:], idxB_in[g])
                dcol = sb.tile([P, K], bf16, tag="dcol")
                nc.sync.dma_start(dcol[:], dstl_col_in[g])
                drep = sb.tile([P, K * P], bf16, tag="drep")
                if os.environ.get("GAT_NOBCAST"):
                    nc.sync.dma_start(drep[:1, :], dstl_row_in[g:g + 1, :])
                else:
                    nc.sync.dma_start(
                        drep[:], dstl_row_in[g:g + 1, :].to_broadcast([P, K * P]))
                erw_f = sb.tile([P, H], f32, tag="erwf")
                nc.sync.dma_start(erw_f[:], er_d[g * P:(g + 1) * P, :])
                erw = sb.tile([P, H], bf16, tag="erw")
                nc.vector.tensor_copy(erw[:], erw_f[:])

                gt = gat.tile([P, K, ELEM], bf16, tag="gt")
                if edgelvl >= 1:
                    nc.gpsimd.dma_gather(
                        out_ap=gt[:, :KA, :], in_ap=tab_full[:S_SPLIT, :],
                        idxs_ap=idxA_t[:], num_idxs=KA * P,
                        num_idxs_reg=KA * P, elem_size=ELEM,
                        single_packet=False, queue_num=(2 * g) % 4)
                    nc.gpsimd.dma_gather(
                        out_ap=gt[:, KA:, :], in_ap=tab_B[:],
                        idxs_ap=idxB_t[:], num_idxs=KB * P,
                        num_idxs_reg=KB * P, elem_size=ELEM,
                        single_packet=False, queue_num=(2 * g + 1) % 4)

                seg = None
                if not os.environ.get("GAT_NOSEG"):
                    seg = psg.tile([P, width], f32, space="PSUM", tag="seg")
                for b in range(nb):
                    k0 = b * BATCH
                    kb = min(BATCH, K - k0)
                    if edgelvl < 2:
                        continue
                    oh = sb.tile([P, BATCH, P], bf16, tag="oh")
                    nc.vector.tensor_tensor(
                        out=oh[:, :kb, :],
                        in0=dcol[:, k0:k0 + kb, None].to_broadcast([P, kb, P]),
                        in1=iota[:, None, :].to_broadcast([P, kb, P]),
                        op=mybir.AluOpType.is_equal)
                    if edgelvl < 3:
                        continue
                    ohT = sb.tile([P, BATCH, P], bf16, tag="ohT")
                    nc.vector.tensor_tensor(
                        out=ohT[:, :kb, :],
                        in0=drep[:, k0 * P:(k0 + kb) * P].rearrange(
                            "p (k j) -> p k j", k=kb),
                        in1=iotac_rep[:, k0 * P:(k0 + kb) * P].rearrange(
                            "p (k j) -> p k j", k=kb),
                        op=mybir.AluOpType.is_equal)
                    if edgelvl < 4:
                        continue
                    erp = psg.tile([P, BATCH, H], f32, space="PSUM", tag="erp")
                    for c in range(kb):
                        nc.tensor.matmul(
                            erp[:, c, :], lhsT=ohT[:, c, :], rhs=erw[:],
                            start=True, stop=True)
                    if edgelvl < 5:
                        continue
                    ev = sb.tile([P, BATCH, H], f32, tag="ev")
                    nc.vector.tensor_tensor(
                        out=ev[:, :kb, :],
                        in0=gt[:, k0:k0 + kb, width - H:width],
                        in1=erp[:, :kb, :], op=mybir.AluOpType.add)
                    nc.vector.scalar_tensor_tensor(
                        out=ev[:, :kb, :], in0=ev[:, :kb, :], scalar=NEG_SLOPE,
                        in1=ev[:, :kb, :],
                        op0=mybir.AluOpType.mult, op1=mybir.AluOpType.max)
                    if edgelvl < 6:
                        continue
                    mt = sb.tile([P, BATCH, width], bf16, tag="mt")
                    nc.scalar.activation(
                        mt[:, :kb, width - H:width], ev[:, :kb, :],
                        mybir.ActivationFunctionType.Exp)
                    nc.vector.tensor_tensor(
                        out=mt[:, :kb, :width - H].rearrange(
                            "p k (h d) -> p k h d", h=H),
                        in0=gt[:, k0:k0 + kb, :width - H].rearrange(
                            "p k (h d) -> p k h d", h=H),
                        in1=mt[:, :kb, width - H:width, None].to_broadcast(
                            [P, kb, H, (width - H) // H]),
                        op=mybir.AluOpType.mult)
                    if edgelvl < 7:
                        continue
                    for c in range(kb):
                        nc.tensor.matmul(
                            seg[:], lhsT=oh[:, c, :], rhs=mt[:, c, :],
                            start=(b == 0 and c == 0),
                            stop=(b == nb - 1 and c == kb - 1))
                if edgelvl >= 7:
                    out_cb(g, seg)

        def l1_out(g, seg):
            dn = sb.tile([P, H], f32, tag="dn")
            nc.vector.tensor_scalar_max(dn[:], seg[:, HD:HD + H], 1e-30)
            rd = sb.tile([P, H], f32, tag="rd")
            nc.vector.reciprocal(rd[:], dn[:])
            ht = sb.tile([P, F_IN], f32, tag="ht")
            nc.vector.tensor_tensor(
                out=ht[:].rearrange("p (h d) -> p h d", h=H),
                in0=seg[:, :HD].rearrange("p (h d) -> p h d", h=H),
                in1=rd[:, :, None].to_broadcast([P, H, D]),
                op=mybir.AluOpType.mult)
            nc.vector.tensor_tensor(
                out=ht[:], in0=ht[:], in1=b1s[:], op=mybir.AluOpType.add)
            mn = sb.tile([P, F_IN], f32, tag="mn")
            nc.vector.tensor_scalar_min(mn[:], ht[:], 0.0)
            nc.scalar.activation(mn[:], mn[:], mybir.ActivationFunctionType.Exp)
            nc.vector.scalar_tensor_tensor(
                out=h1[:, g, :], in0=mn[:], scalar=-1.0, in1=ht[:],
                op0=mybir.AluOpType.add, op1=mybir.AluOpType.max)

        if go("edges1"):
            edge_phase(tab1, tab1B, er1_d, W1, l1_out)
        if phase == "edges1" and edgelvl >= 7:
            for g in range(G):
                hd_t = sb.tile([P, F_IN], bf16, tag="hdump")
                nc.vector.tensor_copy(hd_t[:], h1[:, g, :])
                nc.sync.dma_start(dbg[g * P:(g + 1) * P, :F_IN], hd_t[:])

        def h1_tile(g):
            return h1[:, g, :]

        if go("proj2"):
            project(h1_tile, w2, W2, tab2_own, er2_d)
            nc.gpsimd.collective_compute(
                "AllGather", mybir.AluOpType.bypass,
                replica_groups=[list(range(NCORES))],
                ins=[tab2_own[:]], outs=[tab2[:]])
            nc.sync.dma_start(tab2B[:], tab2[S_SPLIT:, :])

        def l2_out(g, seg):
            dn = sb.tile([P, H], f32, tag="dn2")
            nc.vector.tensor_scalar_max(dn[:], seg[:, HC:HC + H], 1e-30)
            rd = sb.tile([P, H], f32, tag="rd2")
            nc.vector.reciprocal(rd[:], dn[:])
            nc.vector.tensor_scalar_mul(rd[:], rd[:], 1.0 / H)
            z = sb.tile([P, HC], f32, tag="z")
            nc.vector.tensor_tensor(
                out=z[:].rearrange("p (h c) -> p h c", h=H),
                in0=seg[:, :HC].rearrange("p (h c) -> p h c", h=H),
                in1=rd[:, :, None].to_broadcast([P, H, C]),
                op=mybir.AluOpType.mult)
            z4 = sb.tile([P, C], f32, tag="z4")
            nc.vector.reduce_sum(
                z4[:], z[:].rearrange("p (h c) -> p c h", h=H),
                axis=mybir.AxisListType.X)
            nc.vector.tensor_tensor(
                out=z4[:], in0=z4[:], in1=b2ms[:], op=mybir.AluOpType.add)
            zm = sb.tile([P, 1], f32, tag="zm")
            nc.vector.reduce_max(zm[:], z4[:], axis=mybir.AxisListType.X)
            nc.vector.tensor_scalar(
                out=zs[:, g, :], in0=z4[:], scalar1=zm[:, :1], scalar2=None,
                op0=mybir.AluOpType.subtract)
            es = sb.tile([P, C], f32, tag="es")
            nc.scalar.activation(es[:], zs[:, g, :],
                                 mybir.ActivationFunctionType.Exp,
                                 accum_out=ss[:, g:g + 1])

        if go("edges2"):
            edge_phase(tab2, tab2B, er2_d, W2, l2_out)

        if go("final"):
            lg = sb.tile([P, G], f32, tag="lg")
            nc.scalar.activation(lg[:], ss[:], mybir.ActivationFunctionType.Ln)
            for g in range(G):
                yt = sb.tile([P, C], f32, tag="yt")
                nc.vector.tensor_scalar(
                    out=yt[:], in0=zs[:, g, :], scalar1=lg[:, g:g + 1],
                    scalar2=None, op0=mybir.AluOpType.subtract)
                nc.sync.dma_start(y_out[g * P:(g + 1) * P, :], yt[:])

    nc.compile()
    return nc


def kernel(x, src, dst, W1s, W1d, al1, ar1, b1, W2s, W2d, al2, ar2, b2):
    global LAST_EXEC_NS
    x = np.asarray(x, dtype=np.float32)
    src = np.asarray(src, dtype=np.int32)
    dst = np.asarray(dst, dtype=np.int32)

    sch = _schedule(src, dst)
    KA, KB = sch["KA"], sch["KB"]

    def wcat(Ws, Wd, al, ar, hd, h, dim):
        Ws = np.asarray(Ws, np.float32)
        Wd = np.asarray(Wd, np.float32)
        wel = Ws @ _blockdiag(np.asarray(al, np.float32), hd, h, dim)
        wer = Wd @ _blockdiag(np.asarray(ar, np.float32), hd, h, dim)
        return np.concatenate([Ws, wel, wer], axis=1)

    w1c = wcat(W1s, W1d, al1, ar1, HD, H, D).astype(BF16)
    w2c = wcat(W2s, W2d, al2, ar2, HC, H, C).astype(BF16)

    iota_r = np.tile(np.arange(P, dtype=np.float32), (P, 1)).astype(BF16)
    iota_c = np.arange(P, dtype=np.float32)[:, None]
    iotac_rep_np = np.repeat(
        np.arange(P, dtype=np.float32)[:, None], (KA + KB) * P, axis=1
    ).astype(BF16)
    ident_np = np.eye(P, dtype=np.float32)
    b1_np = np.tile(np.asarray(b1, np.float32)[None, :], (P, 1))
    b2m_np = np.tile(np.asarray(b2, np.float32).reshape(H, C).mean(0)[None, :],
                     (P, 1))

    x_pad = np.zeros((NCORES, NPAD, F_IN), np.float32)
    x_pad[:, :NPC, :] = x.reshape(NCORES, NPC, F_IN)

    nc = _build_program(KA, KB)

    in_maps = []
    for c in range(NCORES):
        in_maps.append({
            "x_own": x_pad[c],
            "w1cat": w1c, "w2cat": w2c,
            "b1_t": b1_np, "b2m_t": b2m_np,
            "iota_r": iota_r, "iota_c": iota_c, "ident_t": ident_np,
            "iotac_rep": iotac_rep_np,
            "idxA_in": sch["idxA_w"][c], "idxB_in": sch["idxB_w"][c],
            "dstl_row_in": sch["dstl_row"][c],
            "dstl_col_in": sch["dstl_col"][c],
        })

    res = run_bass_kernel_spmd(nc, in_maps, list(range(NCORES)),
                               trace=bool(os.environ.get("GAT_TRACE")))
    LAST_EXEC_NS = res.exec_time_ns
    out = np.concatenate(
        [res.results[c]["y_out"][:NPC] for c in range(NCORES)], axis=0)
    return out.astype(np.float32)

